# revision 2
# baseline (speedup 1.0000x reference)
"""Trainium2 kernel for nn_BalancedHamiltonLayer — bf16 data path, tuned load order.

Math: out = einsum("btd,rde->bte", x, factors)/sqrt(rank) + bias.
The einsum contracts r as a plain sum, so sum_r (x @ F_r) == x @ (sum_r F_r):
one [16384,2048] @ [2048,2048] GEMM instead of eight.

All operand traffic is bf16 (PE runs bf16 at the same 1 cycle/row as f32r, so
this halves every DMA + the AllGather at no PE cost; PSUM accumulates fp32 and
measured end-to-end rel err is ~4e-3 vs the 2e-2 budget).

Distribution over 8 NeuronCores (single SPMD program):
  - x row-sharded over b*t: core c owns rows [c*2048, (c+1)*2048).
  - factors column-sharded for the reduction: core c reduces
    W_c = sum_r factors[r, :, c*256:(c+1)*256] on-device (DVE tree adds),
    then two AllGathers (one per 128-wide e-half) replicate W. The first
    AllGather unblocks half the GEMM; the second hides under it.
  - GEMM per core: x^T resident in SBUF bf16; W e-tiles [128d x 128e]
    stream through the two HWDGE rings as the stationary operand.
    1/sqrt(8) scale + bias land in the PSUM eviction (DVE tensor_scalar,
    bias per-partition in the transposed layout), output stored bf16.
  - Each core writes out^T [2048e, 2048m]; the host transposes back and
    upcasts to fp32.
"""

import math
import os

import numpy as np
import ml_dtypes

REPEAT = int(os.environ.get("BASS_BENCH_REPEAT", "1"))  # >1 only for benching
B, T, DIM, RANK = 4, 4096, 2048, 8
N_CORES = 8
MC = (B * T) // N_CORES        # 2048 rows per core
EC = DIM // N_CORES            # 256 output cols reduced per core
NT = DIM // 128                # 16 contraction tiles
NET = 2 * N_CORES              # 16 e-tiles of 128 cols
SCALE = 1.0 / math.sqrt(RANK)

BF16 = ml_dtypes.bfloat16
_CACHE = {}


def _build():
    import concourse.bacc as bacc
    import concourse.mybir as mybir
    import concourse.tile as tile

    f32 = mybir.dt.float32
    bf16 = mybir.dt.bfloat16
    add = mybir.AluOpType.add
    mult = mybir.AluOpType.mult
    grp = [list(range(N_CORES))]

    nc = bacc.Bacc(
        "TRN2", target_bir_lowering=False, debug=False, num_devices=N_CORES
    )
    # [j, p, t, m]: x^T chunks, d = t*128+p, m_global = j*512+m
    xh = nc.dram_tensor("xh", [4, 128, NT, 512], bf16, kind="ExternalInput").ap()
    # [eh, rp, p, q, t, e]: this core's factor slice, r = 2*rp+q, d = t*128+p,
    # e_global = 256*core + 128*eh + e
    fh = nc.dram_tensor(
        "fh", [2, RANK // 2, 128, 2, NT, 128], bf16, kind="ExternalInput"
    ).ap()
    # bias_cols[p, et] = bias[r*256 + eh*128 + p] for et = eh*8 + r
    bias_cols = nc.dram_tensor("bias_cols", [128, NET], f32, kind="ExternalInput").ap()
    # transposed output: outT[e, m], bf16
    outT = nc.dram_tensor("outT", [DIM, MC], bf16, kind="ExternalOutput").ap()

    with tile.TileContext(nc) as tc:
        with (
            tc.tile_pool(name="const", bufs=1) as const_pool,
            tc.tile_pool(name="dram", bufs=1, space="DRAM") as dram_pool,
            tc.tile_pool(name="xa", bufs=1) as xa_pool,
        ):
            scope = nc.named_scope
            bias_sb = const_pool.tile([128, NET], f32)
            nc.sync.dma_start(bias_sb[:], bias_cols[:])

            for it in range(REPEAT):
              wc_half = [
                  dram_pool.tile([128, NT, 128], bf16, name=f"wc_half{it}_{i}")
                  for i in range(2)
              ]
              w_half = [
                  dram_pool.tile(
                      [N_CORES, 128, NT, 128], bf16,
                      addr_space="Shared", name=f"w_half{it}_{i}",
                  )
                  for i in range(2)
              ]

              # Phase 1: W_c = sum_r fh[r], per e-half. 4 loads of 1 MB per
              # half, alternating the two HWDGE rings; DVE tree adds; each
              # half's AllGather fires as soon as its sum is stored.
              xa = xa_pool.tile([128, 4, NT, 512], bf16)

              def load_x_chunk(j):
                  # split one 2 MB x chunk across both HWDGE rings
                  nc.sync.dma_start(xa[:, j, 0 : NT // 2], xh[j, :, 0 : NT // 2])
                  nc.scalar.dma_start(xa[:, j, NT // 2 :], xh[j, :, NT // 2 :])

              with (
                  tc.tile_pool(name=f"red{it}", bufs=6) as red_pool,
                  tc.tile_pool(name=f"racc{it}", bufs=1) as acc_pool,
              ):
                  for eh in range(2):
                    with scope(f"reduce{eh}"):
                      pr = []
                      for rp in range(RANK // 2):
                          p_t = red_pool.tile([128, 2, NT, 128], bf16, tag="fr")
                          eng = nc.scalar if rp % 2 == 0 else nc.sync
                          eng.dma_start(p_t[:], fh[eh, rp])  # [p,q,t,e]
                          pr.append(p_t)
                      sA = acc_pool.tile([128, NT, 128], bf16, tag="s0")
                      sB = acc_pool.tile([128, NT, 128], bf16, tag="s1")
                      nc.vector.tensor_add(sA[:], pr[0][:, 0], pr[0][:, 1])
                      nc.vector.tensor_add(sB[:], pr[1][:, 0], pr[1][:, 1])
                      nc.vector.tensor_add(sA[:], sA[:], sB[:])
                      sC = acc_pool.tile([128, NT, 128], bf16, tag="s2")
                      nc.vector.tensor_add(sB[:], pr[2][:, 0], pr[2][:, 1])
                      nc.vector.tensor_add(sC[:], pr[3][:, 0], pr[3][:, 1])
                      nc.vector.tensor_add(sB[:], sB[:], sC[:])
                      sfin = acc_pool.tile([128, NT, 128], bf16, tag="sf")
                      nc.vector.tensor_add(sfin[:], sA[:], sB[:])
                      nc.gpsimd.dma_start(wc_half[eh][:], sfin[:])
                      with scope(f"ag{eh}"):
                          nc.gpsimd.collective_compute(
                              "AllGather", mybir.AluOpType.bypass,
                              ins=[wc_half[eh].opt()],
                              outs=[w_half[eh].opt()],
                              replica_groups=grp,
                          )
                    if eh == 0:
                        # x chunk 0 rides both rings right behind fh[0], so
                        # it is resident well before AG0 lands.
                        load_x_chunk(0)

              load_x_chunk(1)
              # x chunks 2-3 are not consumed until ~50us into the GEMM;
              # holding them off the rings while AllGather 0 is in flight
              # gives the collective full HBM bandwidth on the critical path.
              with tc.tile_wait_until(0.028):
                  for j in range(2, 4):
                      load_x_chunk(j)

              # Phase 3: out^T[e,:] per 128-wide e-tile; W tile stationary,
              # resident x^T streams through the PE.
              with (
                  tc.tile_pool(name=f"wsb{it}", bufs=6) as wpool,
                  tc.tile_pool(name=f"osb{it}", bufs=2) as opool,
                  tc.tile_pool(name=f"ps{it}", bufs=2, space="PSUM") as ppool,
              ):
                  for et in range(NET):
                      eh, r = et // N_CORES, et % N_CORES
                      wsb = wpool.tile([128, NT, 128], bf16, tag="wsb")
                      eng = nc.sync if et % 2 == 0 else nc.scalar
                      eng.dma_start(wsb[:], w_half[eh][r])
                      with scope(f"gemm{et}"):
                          ps = ppool.tile([128, 4, 512], f32, tag="ps")
                          for j in range(4):
                              for t in range(NT):
                                  nc.tensor.matmul(
                                      ps[:, j, :],
                                      wsb[:, t, :],
                                      xa[:, j, t, :],
                                      start=(t == 0),
                                      stop=(t == NT - 1),
                                  )
                          osb = opool.tile([128, MC], bf16, tag="osb")
                          nc.vector.tensor_scalar(
                              osb[:], ps.rearrange("p a b -> p (a b)"),
                              SCALE, bias_sb[:, et : et + 1], mult, add,
                          )
                          e0 = r * EC + eh * 128
                          nc.gpsimd.dma_start(outT[e0 : e0 + 128, :], osb[:])

    nc.compile()
    return nc


def _get_nc():
    if "nc" not in _CACHE:
        _CACHE["nc"] = _build()
    return _CACHE["nc"]


def _shard(x, factors, bias):
    x_flat = np.ascontiguousarray(x, dtype=np.float32).reshape(B * T, DIM)
    factors = np.ascontiguousarray(factors, dtype=np.float32)
    bias = np.ascontiguousarray(bias, dtype=np.float32)
    bias_cols = np.ascontiguousarray(
        bias.reshape(RANK, 2, 128).transpose(2, 1, 0).reshape(128, NET)
    )
    in_maps = []
    for c in range(N_CORES):
        xc = x_flat[c * MC : (c + 1) * MC, :]          # [m, d]
        # -> [j, p, t, m_local] with d = t*128+p, m = j*512+m_local
        xhc = np.ascontiguousarray(
            xc.reshape(4, 512, NT, 128).transpose(0, 3, 2, 1).astype(BF16)
        )
        fc = factors[:, :, c * EC : (c + 1) * EC]       # [r, d, e]
        # -> [eh, rp, p, q, t, e128] with r = 2*rp+q, d = t*128 + p
        fhc = np.ascontiguousarray(
            fc.reshape(RANK // 2, 2, NT, 128, 2, 128)
            .transpose(4, 0, 3, 1, 2, 5)
            .astype(BF16)
        )
        in_maps.append({"xh": xhc, "fh": fhc, "bias_cols": bias_cols})
    return in_maps


def _run(in_maps, trace=False, trace_cores=None):
    from concourse.bass_utils import run_bass_kernel_spmd

    nc = _get_nc()
    return run_bass_kernel_spmd(
        nc, in_maps, list(range(N_CORES)), trace=trace, trace_cores=trace_cores
    )


def _assemble(res):
    out = np.empty((B * T, DIM), dtype=np.float32)
    for c in range(N_CORES):
        out[c * MC : (c + 1) * MC, :] = res.results[c]["outT"].T.astype(np.float32)
    return out.reshape(B, T, DIM)


def kernel(x, factors, bias):
    res = _run(_shard(x, factors, bias), trace=False)
    return _assemble(res)



# revision 8
# speedup vs baseline: 1.3388x; 1.3388x over previous
"""Trainium2 kernel for nn_BalancedHamiltonLayer — no-collective 2x4 sharding.

Math: out = einsum("btd,rde->bte", x, factors)/sqrt(rank) + bias.
The einsum contracts r as a plain sum, so sum_r (x @ F_r) == x @ (sum_r F_r):
one [16384,2048] @ [2048,2048] GEMM instead of eight.

Distribution over 8 NeuronCores (single SPMD program, no collectives):
  core c = (mh, eq) with mh = c//4, eq = c%4 owns
    x rows   [mh*8192, (mh+1)*8192)      (m-sharding x2)
    e-cols   [eq*512, (eq+1)*512)        (e-sharding x4)
  Each core loads its own factor slice [8, 2048, 512] and reduces
  W_c = sum_r F_r[:, eq-slice] on DVE (tree adds) — every core is fully
  independent, so the GEMM starts as soon as the first 4 MB of factors
  lands (~15us) instead of waiting on an AllGather.

GEMM per core ([8192 m] x [2048 d] x [512 e], all bf16, fp32 PSUM):
  W e-tile [128d x 128e] stationary; 4 consecutive m-chunks of 512 share
  each stationary tile, cutting LDWEIGHTS from one-per-matmul (128-cycle
  weight swap per 512 rows = 25% PE overhead) to one per 2048 rows.
  x streams through SBUF in 8 MB quarters (bufs=3); W [2048,512] stays
  resident. PSUM [128,4,512] double-buffered = all 8 banks.
  Epilogue (x*1/sqrt(8) + bias, bias per-partition in the out^T layout)
  runs on the Activation engine so the Vector queue (reduction adds)
  never blocks evictions. Output stored bf16 as out^T [512e, 8192m];
  the host transposes back and upcasts to fp32.
"""

import math
import os

import numpy as np
import ml_dtypes

REPEAT = int(os.environ.get("BASS_BENCH_REPEAT", "1"))  # >1 only for benching
B, T, DIM, RANK = 4, 4096, 2048, 8
N_CORES = 8
MG, EG = 2, 4                  # m-groups x e-groups of cores
MROWS = (B * T) // MG          # 8192 rows per core
EC = DIM // EG                 # 512 e-cols per core
NQ = 4                         # x quarters per core
NMC = (MROWS // NQ) // 512     # 4 m-chunks of 512 per quarter
NT = DIM // 128                # 16 contraction tiles
NET = EC // 128                # 4 e-tiles per core
SCALE = 1.0 / math.sqrt(RANK)

BF16 = ml_dtypes.bfloat16
_CACHE = {}


def _build():
    import concourse.bacc as bacc
    import concourse.mybir as mybir
    import concourse.tile as tile

    f32 = mybir.dt.float32
    bf16 = mybir.dt.bfloat16
    add = mybir.AluOpType.add
    mult = mybir.AluOpType.mult

    nc = bacc.Bacc(
        "TRN2", target_bir_lowering=False, debug=False, num_devices=N_CORES
    )
    # x^T quarters: [q, p, t, mc, m] with d = t*128+p, row = q*2048 + mc*512 + m
    xh = nc.dram_tensor(
        "xh", [NQ, 128, NT, NMC, 512], bf16, kind="ExternalInput"
    ).ap()
    # factor slice: [et, r, p, t, e] with d = t*128+p, e_local = et*128+e
    fh = nc.dram_tensor(
        "fh", [NET, RANK, 128, NT, 128], bf16, kind="ExternalInput"
    ).ap()
    # bias_cols[p, et] = bias[eq*512 + et*128 + p]
    bias_cols = nc.dram_tensor("bias_cols", [128, NET], f32, kind="ExternalInput").ap()
    # transposed output: outT[e_local, m_local], bf16
    outT = nc.dram_tensor("outT", [EC, MROWS], bf16, kind="ExternalOutput").ap()

    with tile.TileContext(nc) as tc:
        with (
            tc.tile_pool(name="const", bufs=1) as const_pool,
            tc.tile_pool(name="xa", bufs=2) as xa_pool,
            tc.tile_pool(name="fr", bufs=8) as fr_pool,
            tc.tile_pool(name="acc", bufs=1) as acc_pool,
            tc.tile_pool(name="w", bufs=4) as w_pool,
            tc.tile_pool(name="osb", bufs=2) as o_pool,
            tc.tile_pool(name="ps", bufs=2, space="PSUM") as p_pool,
        ):
            scope = nc.named_scope
            bias_sb = const_pool.tile([128, NET], f32)
            nc.sync.dma_start(bias_sb[:], bias_cols[:])

            for it in range(REPEAT):
                # ---- load + reduce factors; x quarters issued in priority
                # order between the factor e-tiles so the DMA rings serve
                # the earliest GEMM groups first.
                def issue_fh(et):
                    frs = []
                    for r in range(RANK):
                        t = fr_pool.tile([128, NT, 128], bf16, tag="fr")
                        eng = nc.sync if r % 2 == 0 else nc.scalar
                        eng.dma_start(t[:], fh[et, r])
                        frs.append(t)
                    return frs

                def reduce_et(et, frs):
                    with scope(f"red{it}_{et}"):
                        sA = acc_pool.tile([128, NT, 128], bf16, tag="sA")
                        sB = acc_pool.tile([128, NT, 128], bf16, tag="sB")
                        nc.vector.tensor_add(sA[:], frs[0][:], frs[1][:])
                        nc.vector.tensor_add(sB[:], frs[2][:], frs[3][:])
                        nc.vector.tensor_add(sA[:], sA[:], sB[:])
                        nc.vector.tensor_add(sB[:], frs[4][:], frs[5][:])
                        nc.vector.tensor_add(sA[:], sA[:], sB[:])
                        nc.vector.tensor_add(sB[:], frs[6][:], frs[7][:])
                        w = w_pool.tile([128, NT, 128], bf16, tag="w")
                        nc.vector.tensor_add(w[:], sA[:], sB[:])
                    return w

                def issue_x(q, xa):
                    # 8 pieces of 2 d-tiles (1 MB) alternating the rings so
                    # the d-loop can start as soon as the first piece lands
                    for h in range(8):
                        eng = nc.sync if h % 2 == 0 else nc.scalar
                        eng.dma_start(
                            xa[:, 2 * h : 2 * h + 2], xh[q, :, 2 * h : 2 * h + 2]
                        )

                wts = []
                frs0 = issue_fh(0)
                xa_tiles = [
                    xa_pool.tile(
                        [128, NT, NMC, 512], bf16, tag="xa", name=f"xa{it}_0"
                    )
                ]
                issue_x(0, xa_tiles[0])
                wts.append(reduce_et(0, frs0))
                for et in range(1, NET):
                    wts.append(reduce_et(et, issue_fh(et)))
                for q in range(1, NQ):
                    xa_tiles.append(
                        xa_pool.tile(
                            [128, NT, NMC, 512], bf16, tag="xa", name=f"xa{it}_{q}"
                        )
                    )
                    issue_x(q, xa_tiles[q])

                # ---- GEMM: out^T[e-tile, q-quarter], W stationary, grouped
                # m-chunks amortize the weight swap.
                for q in range(NQ):
                    xa = xa_tiles[q]
                    for et in range(NET):
                        with scope(f"g{it}_{q}_{et}"):
                            ps = p_pool.tile([128, NMC, 512], f32, tag="ps")
                            for d in range(NT):
                                for mc in range(NMC):
                                    nc.tensor.matmul(
                                        ps[:, mc, :],
                                        wts[et][:, d, :],
                                        xa[:, d, mc, :],
                                        start=(d == 0),
                                        stop=(d == NT - 1),
                                    )
                            osb = o_pool.tile([128, NMC * 512], bf16, tag="osb")
                            nc.scalar.activation(
                                osb[:],
                                ps.rearrange("p a b -> p (a b)"),
                                mybir.ActivationFunctionType.Identity,
                                bias=bias_sb[:, et : et + 1],
                                scale=SCALE,
                            )
                            e0 = et * 128
                            m0 = q * NMC * 512
                            nc.gpsimd.dma_start(
                                outT[e0 : e0 + 128, m0 : m0 + NMC * 512], osb[:]
                            )

    nc.compile()
    return nc


def _get_nc():
    if "nc" not in _CACHE:
        _CACHE["nc"] = _build()
    return _CACHE["nc"]


def _shard(x, factors, bias):
    x_flat = np.ascontiguousarray(x, dtype=np.float32).reshape(B * T, DIM)
    factors = np.ascontiguousarray(factors, dtype=np.float32)
    bias = np.ascontiguousarray(bias, dtype=np.float32)
    in_maps = []
    for c in range(N_CORES):
        mh, eq = divmod(c, EG)
        xc = x_flat[mh * MROWS : (mh + 1) * MROWS, :]       # [m, d]
        # -> [q, p, t, mc, m512]
        xhc = np.ascontiguousarray(
            xc.reshape(NQ, NMC, 512, NT, 128).transpose(0, 4, 3, 1, 2).astype(BF16)
        )
        fc = factors[:, :, eq * EC : (eq + 1) * EC]          # [r, d, e]
        # -> [et, r, p, t, e128]
        fhc = np.ascontiguousarray(
            fc.reshape(RANK, NT, 128, NET, 128).transpose(3, 0, 2, 1, 4).astype(BF16)
        )
        bias_c = np.ascontiguousarray(
            bias[eq * EC : (eq + 1) * EC].reshape(NET, 128).T
        )
        in_maps.append({"xh": xhc, "fh": fhc, "bias_cols": bias_c})
    return in_maps


def _run(in_maps, trace=False, trace_cores=None):
    from concourse.bass_utils import run_bass_kernel_spmd

    nc = _get_nc()
    return run_bass_kernel_spmd(
        nc, in_maps, list(range(N_CORES)), trace=trace, trace_cores=trace_cores
    )


def _assemble(res):
    out = np.empty((B * T, DIM), dtype=np.float32)
    for c in range(N_CORES):
        mh, eq = divmod(c, EG)
        out[mh * MROWS : (mh + 1) * MROWS, eq * EC : (eq + 1) * EC] = (
            res.results[c]["outT"].T.astype(np.float32)
        )
    return out.reshape(B, T, DIM)


def kernel(x, factors, bias):
    res = _run(_shard(x, factors, bias), trace=False)
    return _assemble(res)
